# revision 8
# baseline (speedup 1.0000x reference)
"""CPC loss kernel for Trainium2 (8 NeuronCores, SPMD data-parallel over batch N).

Math (per batch element n, handled by core n):
  Az[t]   = W @ latent[n, t]            (K*C = 3072 outputs per position)
  scores[t, k, m] = phi[s_{t,m}] . Az[t, k]   (M=128 negatives per position)
  num[t, k]       = latent[n, 1+t+k] . Az[t, k]
  loss = mean over (n, t<500, k) of log(sum_m exp(scores) + exp(num)) - num

Device strategy per core (v2: DoubleRow score matmuls, 4-bank PSUM megatiles):
  - Host materializes the per-position rhs stream in fp8-e4m3 in the
    [c-half-major] layout the PE needs: phi[p, h, t, 0:12] are the positive
    latent cols (diagonal j==k extracted later), phi[p, h, t, 12:140] the
    gathered negatives.  Only the 500 real positions are streamed (~17.5 MB),
    in 17 chunks with a tiny final chunk so the drain tail is short.
  - AzT is computed via fp8 DoubleRow matmuls in two t-halves (so the second
    half's work overlaps the first score tiles) and stored fp8 in SBUF as
    azk[hh, k, t]; the score lhsT [128, 2, 32] is a strided view of it.
  - Scores: per position ONE DoubleRow matmul (contracts both c-halves) of
    the [128, 2, 140] stream block into a 4-bank PSUM tile (16 positions per
    tile, packed 4-per-bank via tile_position).  Per tile: one ACT
    exp(x-50) over [128, 4, 128] (bf16 out), one DVE masked mul extracting
    num from the 12 positive cols, and two DVE reduces.
  - Final: ln(tot*2^-32) with accum_out row-sum, minus the num row-sum,
    masked partition-sum via 1-col matmul.
Host: loss = sum(partials)/48000 + 50 + 32*ln(2).
"""

import sys, os

for _p in ("/opt/trn_rl_repo", "/root/.axon_site/_ro/trn_rl_repo"):
    if _p not in sys.path:
        sys.path.append(_p)

import numpy as np
import ml_dtypes

import concourse.bass as bass
import concourse.bacc as bacc
import concourse.mybir as mybir
from concourse.tile import TileContext, add_dep_helper

BF16 = ml_dtypes.bfloat16
FP8 = ml_dtypes.float8_e4m3

N, T, C, K, M = 8, 512, 256, 12, 128
Tp = T - K  # 500 real positions
PB = 12  # positive cols per position (k = 0..11)
FB = PB + M  # 140 stream cols per (position, c-half)
SHIFT = 50.0  # fixed logsumexp shift; |scores| << SHIFT + 88 so exp never overflows
DENOM = N * Tp * K  # 48000
NTILE = 32  # score tiles: 31 full (16 positions) + 1 of 4 positions
NV = Tp // 4  # 125 valid tot columns (4 positions each)

# phi chunk t-ranges: 15x32 + 16 + 4 (tiny last chunk -> short drain tail)
CHUNKS = [(32 * i, 32 * (i + 1)) for i in range(15)] + [(480, 496), (496, 500)]


def build_bass():
    nc = bacc.Bacc(
        "TRN2",
        target_bir_lowering=False,
        debug=False,
        enable_asserts=False,
    )
    dt = mybir.dt
    DR = mybir.MatmulPerfMode.DoubleRow

    # phi8[p, t, h, 0:12]   = fp8(latent[n, 1+t+j, h*128 + p]) (positives)
    # phi8[p, t, h, 12:140] = fp8(latent[samps[n,t,m] // T, _ % T, h*128 + p])
    phi8 = nc.dram_tensor("phi8", [128, 2 * Tp * FB], dt.float8e4, kind="ExternalInput").ap()
    latT8 = nc.dram_tensor("latT8", [128, 2, T], dt.float8e4, kind="ExternalInput").ap()
    wT8 = nc.dram_tensor("wT8", [128, 2 * K, 2, 128], dt.float8e4, kind="ExternalInput").ap()
    pmask = nc.dram_tensor("pmask", [128, 1], dt.float32, kind="ExternalInput").ap()
    maskI = nc.dram_tensor("maskI", [128, 4 * PB], dt.float32, kind="ExternalInput").ap()
    out = nc.dram_tensor("out", [1, 1], dt.float32, kind="ExternalOutput").ap()
    phi8v = phi8.rearrange("p (t hh j) -> p t hh j", hh=2, j=FB)

    with TileContext(nc) as tc:
        with (
            tc.tile_pool(name="const", bufs=1) as cp,
            tc.tile_pool(name="phi", bufs=1) as pp,
            tc.tile_pool(name="scr", bufs=6) as sp,
            tc.tile_pool(name="acc", bufs=1) as ap_,
        ):
            # --- constants + small DMAs on the (otherwise idle) SWDGE ring ---
            negshift = cp.tile([128, 1], dt.float32)
            nc.vector.memset(negshift[:], -SHIFT)
            wtile = cp.tile([128, 256], dt.bfloat16)
            nc.vector.memset(wtile[:], 0.5)
            # preload both ACT table sets (exp, ln) so the first real exp /
            # final ln don't stall on a ~1.3us table load
            tldca = cp.tile([128, 1], dt.float32)
            tldcb = cp.tile([128, 1], dt.float32)
            nc.vector.memset(tldca[:], 1.0)
            nc.scalar.activation(out=tldcb[:], in_=tldca[:], func=mybir.ActivationFunctionType.Exp)
            nc.scalar.activation(out=tldcb[:], in_=tldca[:], func=mybir.ActivationFunctionType.Ln)

            pmask_t = cp.tile([128, 1], dt.float32)
            maskI_t = cp.tile([128, 4, PB], dt.float32)
            nc.gpsimd.dma_start(pmask_t[:], pmask[:])
            nc.gpsimd.dma_start(maskI_t[:], maskI[:].rearrange("p (s j) -> p s j", j=PB))

            # --- weight + latent loads first on the sync ring, then the phi
            # stream; everything is consumed in roughly this order -----------
            latT8_t = cp.tile([128, 2, T], dt.float8e4)
            nc.sync.dma_start(latT8_t[:], latT8[:])
            wT8_t = cp.tile([128, 2 * K, 2, 128], dt.float8e4)
            nc.sync.dma_start(wT8_t[:], wT8[:])

            # AzT store: azk[p, hh, k, t] = Az[k, hh*128+p, t] (fp8), k padded
            # to 32 with zeros so the pad output partitions produce zero scores
            azsb = ap_.tile([128, 2 * 32 * T], dt.float8e4)
            azk = azsb.rearrange("p (hh k t) -> p hh k t", hh=2, k=32)
            nc.vector.memzero(azk[:, 0, K:32, :])
            nc.vector.memzero(azk[:, 1, K:32, :])

            # --- phi stream ---------------------------------------------------
            phi_t = pp.tile([128, 2 * Tp * FB], dt.float8e4)
            phi4 = phi_t.rearrange("p (t hh j) -> p t hh j", hh=2, j=FB)
            for (t0, t1) in CHUNKS:
                nc.sync.dma_start(phi4[:, t0:t1], phi8v[:, t0:t1])

            tot_all = ap_.tile([128, 128], dt.bfloat16)
            num_all = ap_.tile([128, 128], dt.float32)

            with tc.tile_pool(name="ps", bufs=2, space="PSUM") as scps:
                # dummy matmuls ramp the PE p-state out of its cold clock
                # while the weight DMAs are still in flight
                W0 = scps.tile([128, 4, 512], dt.float32, name="P")
                for _ in range(26):
                    nc.tensor.matmul(
                        W0[:, 0, 0:128],
                        lhsT=wtile[:, 0:128],
                        rhs=wtile[:, 128:256],
                        start=True,
                        stop=True,
                    )

                # --- Az phase, one t-half at a time ---------------------------
                # AzT[kc, t] = sum_c' W[kc, c'] latent[n, t, c']; one DoubleRow
                # matmul per (k, h) contracts both c'-halves.  Two k's share a
                # 4-bank PSUM tile; each k is copied out fp8 in one strided op.
                def az_half(f):
                    tf = 256 * f
                    for kp in range(K // 2):
                        pa = scps.tile([128, 4, 512], dt.float32, name="P")
                        for i in range(2):
                            k_ = 2 * kp + i
                            for h_ in range(2):
                                nc.tensor.matmul(
                                    pa[:, 2 * i + h_, 0:256],
                                    lhsT=wT8_t[:, 2 * k_ + h_, :, :],
                                    rhs=latT8_t[:, :, tf : tf + 256],
                                    start=True,
                                    stop=True,
                                    perf_mode=DR,
                                )
                        for i in range(2):
                            k_ = 2 * kp + i
                            dst = azk[:, :, k_, tf : tf + 256]
                            src = pa[:, 2 * i : 2 * i + 2, 0:256]
                            if kp % 2 == i % 2:
                                nc.scalar.copy(out=dst, in_=src)
                            else:
                                nc.vector.tensor_copy(out=dst, in_=src)

                # --- one score tile: 16 positions, 4 PSUM banks ---------------
                def score_tile(j):
                    nb = 4 if j < NTILE - 1 else 1
                    P = scps.tile([128, 4, 512], dt.float32, name="P")
                    for s in range(nb):
                        tb = 16 * j + 4 * s
                        # q=0 position via DoubleRow (walrus only accepts DR
                        # at tile_position (0,0)); issued FIRST so that if its
                        # col_grp engages the 32:64 quadrant, q=1's start=True
                        # matmul overwrites any spill.
                        nc.tensor.matmul(
                            P[0:32, s, 0:FB],
                            lhsT=azk[:, :, :, tb],
                            rhs=phi4[:, tb, :, :],
                            start=True,
                            stop=True,
                            perf_mode=DR,
                            tile_position=(0, 0),
                        )
                        for q in range(1, 4):
                            t = tb + q
                            for h in range(2):
                                nc.tensor.matmul(
                                    P[32 * q : 32 * q + 32, s, 0:FB],
                                    lhsT=azk[:, h, :, t],
                                    rhs=phi4[:, t, h, :],
                                    start=(h == 0),
                                    stop=(h == 1),
                                    tile_position=(0, 32 * q),
                                )
                    # exp(scores - 50) for the whole tile in one ACT op
                    E = sp.tile([128, 4, M], dt.bfloat16, tag="exp", name="exp_o")
                    nc.scalar.activation(
                        out=E[:, 0:nb, :],
                        in_=P[:, 0:nb, PB:FB],
                        func=mybir.ActivationFunctionType.Exp,
                        bias=negshift[:],
                        scale=1.0,
                    )
                    # num extraction (diagonal j==k of the positive cols)
                    scr = sp.tile([128, 4, PB], dt.float32, tag="ttr", name="ttr_o")
                    nc.vector.tensor_mul(
                        scr[:, 0:nb, :], P[:, 0:nb, 0:PB], maskI_t[:, 0:nb, :]
                    )
                    nc.vector.tensor_reduce(
                        num_all[:, 4 * j : 4 * j + nb],
                        scr[:, 0:nb, :],
                        axis=mybir.AxisListType.X,
                        op=mybir.AluOpType.add,
                    )
                    with nc.allow_low_precision(reason="bf16 tot validated <1e-5"):
                        nc.vector.tensor_reduce(
                            tot_all[:, 4 * j : 4 * j + nb],
                            E[:, 0:nb, :],
                            axis=mybir.AxisListType.X,
                            op=mybir.AluOpType.add,
                        )

                az_half(0)
                score_tile(0)
                az_half(1)
                for j in range(1, NTILE):
                    score_tile(j)

                # --- final reduction ------------------------------------------
                # row-sum of num (DVE) runs concurrently with exp(num-50) (ACT)
                numsum = ap_.tile([128, 1], dt.float32)
                nc.vector.tensor_reduce(
                    numsum[:],
                    num_all[:, :NV],
                    axis=mybir.AxisListType.X,
                    op=mybir.AluOpType.add,
                )
                # fold in the positive term for all valid tiles: tot += exp(num-50)
                en_t = ap_.tile([128, NV], dt.bfloat16)
                nc.scalar.activation(
                    out=en_t[:],
                    in_=num_all[:, :NV],
                    func=mybir.ActivationFunctionType.Exp,
                    bias=negshift[:],
                    scale=1.0,
                )
                nc.vector.tensor_add(tot_all[:, :NV], tot_all[:, :NV], en_t[:])
                # ln(tot * 2^-32) keeps the ACT-ln input within its valid range;
                # +32*ln2 is restored on the host.  accum_out row-sums the ln.
                Lt = ap_.tile([128, NV], dt.float32)
                lnsum = ap_.tile([128, 1], dt.float32)
                nc.scalar.activation(
                    out=Lt[:],
                    in_=tot_all[:, :NV],
                    func=mybir.ActivationFunctionType.Ln,
                    scale=float(2.0**-32),
                    accum_out=lnsum[:],
                )
                rs = ap_.tile([128, 1], dt.float32)
                nc.vector.tensor_sub(rs[:], lnsum[:], numsum[:])
                psf = scps.tile([1, 1], dt.float32, name="P")
                nc.tensor.matmul(psf[:], lhsT=rs[:], rhs=pmask_t[:])
                outsb = ap_.tile([1, 1], dt.float32)
                nc.scalar.copy(out=outsb[:], in_=psf[:])
                nc.sync.dma_start(out[:], outsb[:])

    nc.compile()
    return nc


def prep_inputs(latent, W, samps):
    """Host-side sharding + layout marshalling. Returns per-core input maps."""
    latent = np.asarray(latent, dtype=np.float32)
    W = np.asarray(W, dtype=np.float32)
    samps = np.asarray(samps).astype(np.int64).reshape(N, Tp, M)

    lat8_all = latent.reshape(N * T, C).astype(FP8)
    # wT8[p, b, h, j] = W[b*128 + j, h*128 + p]
    wT8 = np.ascontiguousarray(
        W.astype(FP8).reshape(2 * K, 128, 2, 128).transpose(3, 0, 2, 1)
    )
    pmask = ((np.arange(128) % 32) < K).astype(np.float32).reshape(128, 1)
    k_arr = np.arange(128) % 32
    maskD = (
        (np.arange(PB)[None, :] == k_arr[:, None]) & (k_arr < K)[:, None]
    ).astype(np.float32)
    maskI4 = np.ascontiguousarray(np.tile(maskD, (1, 4)))

    win_idx = 1 + np.arange(Tp)[:, None] + np.arange(PB)[None, :]  # (Tp, PB)
    in_maps = []
    for n in range(N):
        lat8_n = lat8_all[n * T : (n + 1) * T]  # (T, C) fp8
        latT8 = np.ascontiguousarray(lat8_n.reshape(T, 2, 128).transpose(2, 1, 0))
        # stream block per (h, t): 12 positive cols ++ 128 gathered negatives
        blk = np.empty((Tp, FB, C), dtype=FP8)
        blk[:, :PB] = lat8_n[win_idx]  # (Tp, PB, C)
        blk[:, PB:] = lat8_all[samps[n]]  # (Tp, M, C)
        # phi8[p, t, h, j] = blk[t, j, h*128+p]
        phi8 = blk.reshape(Tp, FB, 2, 128).transpose(3, 0, 2, 1)
        in_maps.append(
            {
                "phi8": np.ascontiguousarray(phi8.reshape(128, 2 * Tp * FB)),
                "latT8": latT8,
                "wT8": wT8,
                "pmask": pmask,
                "maskI": maskI4,
            }
        )
    return in_maps


_NC_CACHE = None


def kernel(latent, W, samps):
    global _NC_CACHE
    from concourse import bass_utils

    if _NC_CACHE is None:
        _NC_CACHE = build_bass()
    nc = _NC_CACHE
    in_maps = prep_inputs(latent, W, samps)
    res = bass_utils.run_bass_kernel_spmd(nc, in_maps, core_ids=list(range(N)))
    partial = sum(float(r["out"][0, 0]) for r in res.results)
    import math

    return np.float32(partial / DENOM + SHIFT + 32.0 * math.log(2.0))


# revision 9
# speedup vs baseline: 1.3033x; 1.3033x over previous
"""CPC loss kernel for Trainium2 (8 NeuronCores, SPMD data-parallel over batch N).

Math (per batch element n, handled by core n):
  Az[t]   = W @ latent[n, t]            (K*C = 3072 outputs per position)
  scores[t, k, m] = phi[s_{t,m}] . Az[t, k]   (M=128 negatives per position)
  num[t, k]       = latent[n, 1+t+k] . Az[t, k]
  loss = mean over (n, t<500, k) of log(sum_m exp(scores) + exp(num)) - num

Device strategy per core (v2: DoubleRow score matmuls, 4-bank PSUM megatiles):
  - Host materializes the per-position rhs stream in fp8-e4m3 in the
    [c-half-major] layout the PE needs: phi[p, h, t, 0:12] are the positive
    latent cols (diagonal j==k extracted later), phi[p, h, t, 12:140] the
    gathered negatives.  Only the 500 real positions are streamed (~17.5 MB),
    in 17 chunks with a tiny final chunk so the drain tail is short.
  - AzT is computed via fp8 DoubleRow matmuls in two t-halves (so the second
    half's work overlaps the first score tiles) and stored fp8 in SBUF as
    azk[hh, k, t]; the score lhsT [128, 2, 32] is a strided view of it.
  - Scores: per position ONE DoubleRow matmul (contracts both c-halves) of
    the [128, 2, 140] stream block into a 4-bank PSUM tile (16 positions per
    tile, packed 4-per-bank via tile_position).  Per tile: one ACT
    exp(x-50) over [128, 4, 128] (bf16 out), one DVE masked mul extracting
    num from the 12 positive cols, and two DVE reduces.
  - Final: ln(tot*2^-32) with accum_out row-sum, minus the num row-sum,
    masked partition-sum via 1-col matmul.
Host: loss = sum(partials)/48000 + 50 + 32*ln(2).
"""

import sys, os

for _p in ("/opt/trn_rl_repo", "/root/.axon_site/_ro/trn_rl_repo"):
    if _p not in sys.path:
        sys.path.append(_p)

import numpy as np
import ml_dtypes

import concourse.bass as bass
import concourse.bacc as bacc
import concourse.mybir as mybir
from concourse.tile import TileContext, add_dep_helper

BF16 = ml_dtypes.bfloat16
FP8 = ml_dtypes.float8_e4m3

N, T, C, K, M = 8, 512, 256, 12, 128
Tp = T - K  # 500 real positions
PB = 12  # positive cols per position (k = 0..11)
FB = PB + M  # 140 stream cols per (position, c-half)
SHIFT = 50.0  # fixed logsumexp shift; |scores| << SHIFT + 88 so exp never overflows
DENOM = N * Tp * K  # 48000
NTILE = 32  # score tiles: 31 full (16 positions) + 1 of 4 positions
NV = Tp // 4  # 125 valid tot columns (4 positions each)

# phi chunk t-ranges: 15x32 + 16 + 4 (tiny last chunk -> short drain tail)
CHUNKS = [(32 * i, 32 * (i + 1)) for i in range(15)] + [(480, 496), (496, 500)]


def build_bass():
    nc = bacc.Bacc(
        "TRN2",
        target_bir_lowering=False,
        debug=False,
        enable_asserts=False,
    )
    dt = mybir.dt
    DR = mybir.MatmulPerfMode.DoubleRow

    # phi8[p, t, h, 0:12]   = fp8(latent[n, 1+t+j, h*128 + p]) (positives)
    # phi8[p, t, h, 12:140] = fp8(latent[samps[n,t,m] // T, _ % T, h*128 + p])
    phi8 = nc.dram_tensor("phi8", [128, 2 * Tp * FB], dt.float8e4, kind="ExternalInput").ap()
    latT8 = nc.dram_tensor("latT8", [128, 2, T], dt.float8e4, kind="ExternalInput").ap()
    wT8 = nc.dram_tensor("wT8", [128, 2 * K, 2, 128], dt.float8e4, kind="ExternalInput").ap()
    pmask = nc.dram_tensor("pmask", [128, 1], dt.float32, kind="ExternalInput").ap()
    maskI = nc.dram_tensor("maskI", [128, 4 * PB], dt.float32, kind="ExternalInput").ap()
    out = nc.dram_tensor("out", [1, 1], dt.float32, kind="ExternalOutput").ap()
    phi8v = phi8.rearrange("p (t hh j) -> p t hh j", hh=2, j=FB)

    with TileContext(nc) as tc:
        with (
            tc.tile_pool(name="const", bufs=1) as cp,
            tc.tile_pool(name="phi", bufs=1) as pp,
            tc.tile_pool(name="scr", bufs=6) as sp,
            tc.tile_pool(name="acc", bufs=1) as ap_,
        ):
            # --- constants + small DMAs on the (otherwise idle) SWDGE ring ---
            negshift = cp.tile([128, 1], dt.float32)
            nc.vector.memset(negshift[:], -SHIFT)
            wtile = cp.tile([128, 256], dt.bfloat16)
            nc.vector.memset(wtile[:], 0.5)
            # preload both ACT table sets (exp, ln) so the first real exp /
            # final ln don't stall on a ~1.3us table load
            tldca = cp.tile([128, 1], dt.float32)
            tldcb = cp.tile([128, 1], dt.float32)
            nc.vector.memset(tldca[:], 1.0)
            nc.scalar.activation(out=tldcb[:], in_=tldca[:], func=mybir.ActivationFunctionType.Exp)
            nc.scalar.activation(out=tldcb[:], in_=tldca[:], func=mybir.ActivationFunctionType.Ln)

            pmask_t = cp.tile([128, 1], dt.float32)
            maskI_t = cp.tile([128, 4, PB], dt.float32)
            nc.gpsimd.dma_start(pmask_t[:], pmask[:])
            nc.gpsimd.dma_start(maskI_t[:], maskI[:].rearrange("p (s j) -> p s j", j=PB))

            # --- weight + latent loads first on the sync ring, then the phi
            # stream; everything is consumed in roughly this order -----------
            latT8_t = cp.tile([128, 2, T], dt.float8e4)
            nc.sync.dma_start(latT8_t[:], latT8[:])
            wT8_t = cp.tile([128, 2 * K, 2, 128], dt.float8e4)
            nc.sync.dma_start(wT8_t[:], wT8[:])

            # AzT store: azk[p, hh, k, t] = Az[k, hh*128+p, t] (fp8), k padded
            # to 32 with zeros so the pad output partitions produce zero scores
            azsb = ap_.tile([128, 2 * 32 * T], dt.float8e4)
            azk = azsb.rearrange("p (hh k t) -> p hh k t", hh=2, k=32)
            nc.vector.memzero(azk[:, 0, K:32, :])
            nc.vector.memzero(azk[:, 1, K:32, :])

            # --- phi stream ---------------------------------------------------
            phi_t = pp.tile([128, 2 * Tp * FB], dt.float8e4)
            phi4 = phi_t.rearrange("p (t hh j) -> p t hh j", hh=2, j=FB)
            for (t0, t1) in CHUNKS:
                nc.sync.dma_start(phi4[:, t0:t1], phi8v[:, t0:t1])

            tot_all = ap_.tile([128, 128], dt.bfloat16)
            num_all = ap_.tile([128, 128], dt.float32)

            with tc.tile_pool(name="ps", bufs=2, space="PSUM") as scps:
                # dummy matmuls ramp the PE p-state out of its cold clock
                # while the weight DMAs are still in flight
                W0 = scps.tile([128, 4, 512], dt.float32, name="P")
                for _ in range(26):
                    nc.tensor.matmul(
                        W0[:, 0, 0:128],
                        lhsT=wtile[:, 0:128],
                        rhs=wtile[:, 128:256],
                        start=True,
                        stop=True,
                    )

                # --- Az phase, one t-half at a time ---------------------------
                # AzT[kc, t] = sum_c' W[kc, c'] latent[n, t, c']; one DoubleRow
                # matmul per (k, h) contracts both c'-halves.  Two k's share a
                # 4-bank PSUM tile; each k is copied out fp8 in one strided op.
                def az_half(f):
                    tf = 256 * f
                    for kp in range(K // 2):
                        pa = scps.tile([128, 4, 512], dt.float32, name="P")
                        for i in range(2):
                            k_ = 2 * kp + i
                            for h_ in range(2):
                                nc.tensor.matmul(
                                    pa[:, 2 * i + h_, 0:256],
                                    lhsT=wT8_t[:, 2 * k_ + h_, :, :],
                                    rhs=latT8_t[:, :, tf : tf + 256],
                                    start=True,
                                    stop=True,
                                    perf_mode=DR,
                                )
                        for i in range(2):
                            k_ = 2 * kp + i
                            dst = azk[:, :, k_, tf : tf + 256]
                            src = pa[:, 2 * i : 2 * i + 2, 0:256]
                            if kp % 2 == i % 2:
                                nc.scalar.copy(out=dst, in_=src)
                            else:
                                nc.vector.tensor_copy(out=dst, in_=src)

                # --- one score tile: 16 positions, 4 PSUM banks ---------------
                def score_tile(j):
                    nb = 4 if j < NTILE - 1 else 1
                    P = scps.tile([128, 4, 512], dt.float32, name="P")
                    for s in range(nb):
                        for q in range(4):
                            t = 16 * j + 4 * s + q
                            for h in range(2):
                                nc.tensor.matmul(
                                    P[32 * q : 32 * q + 32, s, 0:FB],
                                    lhsT=azk[:, h, :, t],
                                    rhs=phi4[:, t, h, :],
                                    start=(h == 0),
                                    stop=(h == 1),
                                    tile_position=(0, 32 * q),
                                )
                    # exp(scores - 50) for the whole tile in one ACT op
                    E = sp.tile([128, 4, M], dt.bfloat16, tag="exp", name="exp_o")
                    nc.scalar.activation(
                        out=E[:, 0:nb, :],
                        in_=P[:, 0:nb, PB:FB],
                        func=mybir.ActivationFunctionType.Exp,
                        bias=negshift[:],
                        scale=1.0,
                    )
                    # num extraction (diagonal j==k of the positive cols)
                    scr = sp.tile([128, 4, PB], dt.float32, tag="ttr", name="ttr_o")
                    nc.vector.tensor_mul(
                        scr[:, 0:nb, :], P[:, 0:nb, 0:PB], maskI_t[:, 0:nb, :]
                    )
                    nc.vector.tensor_reduce(
                        num_all[:, 4 * j : 4 * j + nb],
                        scr[:, 0:nb, :],
                        axis=mybir.AxisListType.X,
                        op=mybir.AluOpType.add,
                    )
                    with nc.allow_low_precision(reason="bf16 tot validated <1e-5"):
                        nc.vector.tensor_reduce(
                            tot_all[:, 4 * j : 4 * j + nb],
                            E[:, 0:nb, :],
                            axis=mybir.AxisListType.X,
                            op=mybir.AluOpType.add,
                        )

                az_half(0)
                score_tile(0)
                az_half(1)
                for j in range(1, NTILE):
                    score_tile(j)

                # --- final reduction ------------------------------------------
                # row-sum of num (DVE) runs concurrently with exp(num-50) (ACT)
                numsum = ap_.tile([128, 1], dt.float32)
                nc.vector.tensor_reduce(
                    numsum[:],
                    num_all[:, :NV],
                    axis=mybir.AxisListType.X,
                    op=mybir.AluOpType.add,
                )
                # fold in the positive term for all valid tiles: tot += exp(num-50)
                en_t = ap_.tile([128, NV], dt.bfloat16)
                nc.scalar.activation(
                    out=en_t[:],
                    in_=num_all[:, :NV],
                    func=mybir.ActivationFunctionType.Exp,
                    bias=negshift[:],
                    scale=1.0,
                )
                nc.vector.tensor_add(tot_all[:, :NV], tot_all[:, :NV], en_t[:])
                # ln(tot * 2^-32) keeps the ACT-ln input within its valid range;
                # +32*ln2 is restored on the host.  accum_out row-sums the ln.
                Lt = ap_.tile([128, NV], dt.float32)
                lnsum = ap_.tile([128, 1], dt.float32)
                nc.scalar.activation(
                    out=Lt[:],
                    in_=tot_all[:, :NV],
                    func=mybir.ActivationFunctionType.Ln,
                    scale=float(2.0**-32),
                    accum_out=lnsum[:],
                )
                rs = ap_.tile([128, 1], dt.float32)
                nc.vector.tensor_sub(rs[:], lnsum[:], numsum[:])
                psf = scps.tile([1, 1], dt.float32, name="P")
                nc.tensor.matmul(psf[:], lhsT=rs[:], rhs=pmask_t[:])
                outsb = ap_.tile([1, 1], dt.float32)
                nc.scalar.copy(out=outsb[:], in_=psf[:])
                nc.sync.dma_start(out[:], outsb[:])

    nc.compile()
    return nc


def prep_inputs(latent, W, samps):
    """Host-side sharding + layout marshalling. Returns per-core input maps."""
    latent = np.asarray(latent, dtype=np.float32)
    W = np.asarray(W, dtype=np.float32)
    samps = np.asarray(samps).astype(np.int64).reshape(N, Tp, M)

    lat8_all = latent.reshape(N * T, C).astype(FP8)
    # wT8[p, b, h, j] = W[b*128 + j, h*128 + p]
    wT8 = np.ascontiguousarray(
        W.astype(FP8).reshape(2 * K, 128, 2, 128).transpose(3, 0, 2, 1)
    )
    pmask = ((np.arange(128) % 32) < K).astype(np.float32).reshape(128, 1)
    k_arr = np.arange(128) % 32
    maskD = (
        (np.arange(PB)[None, :] == k_arr[:, None]) & (k_arr < K)[:, None]
    ).astype(np.float32)
    maskI4 = np.ascontiguousarray(np.tile(maskD, (1, 4)))

    win_idx = 1 + np.arange(Tp)[:, None] + np.arange(PB)[None, :]  # (Tp, PB)
    in_maps = []
    for n in range(N):
        lat8_n = lat8_all[n * T : (n + 1) * T]  # (T, C) fp8
        latT8 = np.ascontiguousarray(lat8_n.reshape(T, 2, 128).transpose(2, 1, 0))
        # stream block per (h, t): 12 positive cols ++ 128 gathered negatives
        blk = np.empty((Tp, FB, C), dtype=FP8)
        blk[:, :PB] = lat8_n[win_idx]  # (Tp, PB, C)
        blk[:, PB:] = lat8_all[samps[n]]  # (Tp, M, C)
        # phi8[p, t, h, j] = blk[t, j, h*128+p]
        phi8 = blk.reshape(Tp, FB, 2, 128).transpose(3, 0, 2, 1)
        in_maps.append(
            {
                "phi8": np.ascontiguousarray(phi8.reshape(128, 2 * Tp * FB)),
                "latT8": latT8,
                "wT8": wT8,
                "pmask": pmask,
                "maskI": maskI4,
            }
        )
    return in_maps


_NC_CACHE = None


def kernel(latent, W, samps):
    global _NC_CACHE
    from concourse import bass_utils

    if _NC_CACHE is None:
        _NC_CACHE = build_bass()
    nc = _NC_CACHE
    in_maps = prep_inputs(latent, W, samps)
    res = bass_utils.run_bass_kernel_spmd(nc, in_maps, core_ids=list(range(N)))
    partial = sum(float(r["out"][0, 0]) for r in res.results)
    import math

    return np.float32(partial / DENOM + SHIFT + 32.0 * math.log(2.0))


# revision 11
# speedup vs baseline: 1.5414x; 1.1827x over previous
"""CPC loss kernel for Trainium2 (8 NeuronCores, SPMD data-parallel over batch N).

Math (per batch element n, handled by core n):
  Az[t]   = W @ latent[n, t]            (K*C = 3072 outputs per position)
  scores[t, k, m] = phi[s_{t,m}] . Az[t, k]   (M=128 negatives per position)
  num[t, k]       = latent[n, 1+t+k] . Az[t, k]
  loss = mean over (n, t<500, k) of log(sum_m exp(scores) + exp(num)) - num

Device strategy per core (v2: DoubleRow score matmuls, 4-bank PSUM megatiles):
  - Host materializes the per-position rhs stream in fp8-e4m3 in the
    [c-half-major] layout the PE needs: phi[p, h, t, 0:12] are the positive
    latent cols (diagonal j==k extracted later), phi[p, h, t, 12:140] the
    gathered negatives.  Only the 500 real positions are streamed (~17.5 MB),
    in 17 chunks with a tiny final chunk so the drain tail is short.
  - AzT is computed via fp8 DoubleRow matmuls in two t-halves (so the second
    half's work overlaps the first score tiles) and stored fp8 in SBUF as
    azk[hh, k, t]; the score lhsT [128, 2, 32] is a strided view of it.
  - Scores: per position ONE DoubleRow matmul (contracts both c-halves) of
    the [128, 2, 140] stream block into a 4-bank PSUM tile (16 positions per
    tile, packed 4-per-bank via tile_position).  Per tile: one ACT
    exp(x-50) over [128, 4, 128] (bf16 out), one DVE masked mul extracting
    num from the 12 positive cols, and two DVE reduces.
  - Final: ln(tot*2^-32) with accum_out row-sum, minus the num row-sum,
    masked partition-sum via 1-col matmul.
Host: loss = sum(partials)/48000 + 50 + 32*ln(2).
"""

import sys, os

for _p in ("/opt/trn_rl_repo", "/root/.axon_site/_ro/trn_rl_repo"):
    if _p not in sys.path:
        sys.path.append(_p)

import numpy as np
import ml_dtypes

import concourse.bass as bass
import concourse.bacc as bacc
import concourse.mybir as mybir
from concourse.tile import TileContext, add_dep_helper

BF16 = ml_dtypes.bfloat16
FP8 = ml_dtypes.float8_e4m3

N, T, C, K, M = 8, 512, 256, 12, 128
Tp = T - K  # 500 real positions
PB = 12  # positive cols per position (k = 0..11)
FB = PB + M  # 140 stream cols per (position, c-half)
SHIFT = 50.0  # fixed logsumexp shift; |scores| << SHIFT + 88 so exp never overflows
DENOM = N * Tp * K  # 48000
NTILE = 32  # score tiles: 31 full (16 positions) + 1 of 4 positions
NV = Tp // 4  # 125 valid tot columns (4 positions each)

# phi chunk t-ranges: 15x32 + 16 + 4 (tiny last chunk -> short drain tail)
CHUNKS = [(32 * i, 32 * (i + 1)) for i in range(15)] + [(480, 496), (496, 500)]


def build_bass():
    nc = bacc.Bacc(
        "TRN2",
        target_bir_lowering=False,
        debug=False,
        enable_asserts=False,
    )
    dt = mybir.dt
    DR = mybir.MatmulPerfMode.DoubleRow

    # phi8[p, t, h, 0:12]   = fp8(latent[n, 1+t+j, h*128 + p]) (positives)
    # phi8[p, t, h, 12:140] = fp8(latent[samps[n,t,m] // T, _ % T, h*128 + p])
    phi8 = nc.dram_tensor("phi8", [128, 2 * Tp * FB], dt.float8e4, kind="ExternalInput").ap()
    latT8 = nc.dram_tensor("latT8", [128, 2, T], dt.float8e4, kind="ExternalInput").ap()
    wT8 = nc.dram_tensor("wT8", [128, 2 * K, 2, 128], dt.float8e4, kind="ExternalInput").ap()
    pmask = nc.dram_tensor("pmask", [128, 1], dt.float32, kind="ExternalInput").ap()
    maskI = nc.dram_tensor("maskI", [128, 4 * PB], dt.float32, kind="ExternalInput").ap()
    out = nc.dram_tensor("out", [1, 1], dt.float32, kind="ExternalOutput").ap()
    phi8v = phi8.rearrange("p (t hh j) -> p t hh j", hh=2, j=FB)

    with TileContext(nc) as tc:
        with (
            tc.tile_pool(name="const", bufs=1) as cp,
            tc.tile_pool(name="phi", bufs=1) as pp,
            tc.tile_pool(name="scr", bufs=6) as sp,
            tc.tile_pool(name="acc", bufs=1) as ap_,
        ):
            # --- constants + small DMAs on the (otherwise idle) SWDGE ring ---
            negshift = cp.tile([128, 1], dt.float32)
            nc.vector.memset(negshift[:], -SHIFT)
            wtile = cp.tile([128, 256], dt.bfloat16)
            nc.vector.memset(wtile[:], 0.5)
            # preload both ACT table sets (exp, ln) so the first real exp /
            # final ln don't stall on a ~1.3us table load
            tldca = cp.tile([128, 1], dt.float32)
            tldcb = cp.tile([128, 1], dt.float32)
            nc.vector.memset(tldca[:], 1.0)
            nc.scalar.activation(out=tldcb[:], in_=tldca[:], func=mybir.ActivationFunctionType.Exp)
            nc.scalar.activation(out=tldcb[:], in_=tldca[:], func=mybir.ActivationFunctionType.Ln)

            pmask_t = cp.tile([128, 1], dt.float32)
            maskI_t = cp.tile([128, 4, PB], dt.float32)
            nc.gpsimd.dma_start(pmask_t[:], pmask[:])
            nc.gpsimd.dma_start(maskI_t[:], maskI[:].rearrange("p (s j) -> p s j", j=PB))

            # --- weight + latent loads first on the sync ring, then the phi
            # stream; everything is consumed in roughly this order -----------
            latT8_t = cp.tile([128, 2, T], dt.float8e4)
            nc.sync.dma_start(latT8_t[:], latT8[:])
            # wT8 in 6 chunks so the first Az matmuls start as soon as the
            # first k-pairs land
            wT8_t = cp.tile([128, 2 * K, 2, 128], dt.float8e4)
            for c in range(6):
                nc.sync.dma_start(wT8_t[:, 4 * c : 4 * c + 4], wT8[:, 4 * c : 4 * c + 4])

            # AzT store: azk[p, hh, k, t] = Az[k, hh*128+p, t] (fp8), k padded
            # to 32 with zeros so the pad output partitions produce zero scores
            azsb = ap_.tile([128, 2 * 32 * T], dt.float8e4)
            azk = azsb.rearrange("p (hh k t) -> p hh k t", hh=2, k=32)
            nc.vector.memzero(azk[:, 0, K:32, :])
            nc.vector.memzero(azk[:, 1, K:32, :])

            # --- phi stream ---------------------------------------------------
            phi_t = pp.tile([128, 2 * Tp * FB], dt.float8e4)
            phi4 = phi_t.rearrange("p (t hh j) -> p t hh j", hh=2, j=FB)
            for (t0, t1) in CHUNKS:
                nc.sync.dma_start(phi4[:, t0:t1], phi8v[:, t0:t1])

            tot_all = ap_.tile([128, 128], dt.bfloat16)
            num_all = ap_.tile([128, 128], dt.float32)

            with tc.tile_pool(name="ps", bufs=4, space="PSUM") as scps:
                # dummy matmuls ramp the PE p-state out of its cold clock
                # while the weight DMAs are still in flight
                W0 = scps.tile([128, 2, 512], dt.float32, name="P")
                for _ in range(26):
                    nc.tensor.matmul(
                        W0[:, 0, 0:128],
                        lhsT=wtile[:, 0:128],
                        rhs=wtile[:, 128:256],
                        start=True,
                        stop=True,
                    )

                # --- Az phase ------------------------------------------------
                # AzT[kc, t] = sum_c' W[kc, c'] latent[n, t, c']; one DoubleRow
                # matmul per (k, h) contracts both c'-halves; both h-halves of
                # a k land in one 2-bank PSUM tile, copied out fp8 in one op.
                for k_ in range(K):
                    pa = scps.tile([128, 2, T], dt.float32, name="P")
                    for h_ in range(2):
                        nc.tensor.matmul(
                            pa[:, h_, :],
                            lhsT=wT8_t[:, 2 * k_ + h_, :, :],
                            rhs=latT8_t[:, :, :],
                            start=True,
                            stop=True,
                            perf_mode=DR,
                        )
                    if k_ % 2 == 0:
                        nc.scalar.copy(out=azk[:, :, k_, :], in_=pa[:, :, :])
                    else:
                        nc.vector.tensor_copy(out=azk[:, :, k_, :], in_=pa[:, :, :])

                # --- score megatiles: 8 positions, 2 PSUM banks ---------------
                NMT = Tp // 8 + 1  # 62 full + 1 single-bank (positions 496-499)
                for m in range(NMT):
                    nb = 2 if m < NMT - 1 else 1
                    P = scps.tile([128, 2, 512], dt.float32, name="P")
                    for s in range(nb):
                        for q in range(4):
                            t = 8 * m + 4 * s + q
                            for h in range(2):
                                nc.tensor.matmul(
                                    P[32 * q : 32 * q + 32, s, 0:FB],
                                    lhsT=azk[:, h, :, t],
                                    rhs=phi4[:, t, h, :],
                                    start=(h == 0),
                                    stop=(h == 1),
                                    tile_position=(0, 32 * q),
                                )
                    # exp(scores - 50), bf16 out
                    E = sp.tile([128, 2, M], dt.bfloat16, tag="exp", name="exp_o")
                    nc.scalar.activation(
                        out=E[:, 0:nb, :],
                        in_=P[:, 0:nb, PB:FB],
                        func=mybir.ActivationFunctionType.Exp,
                        bias=negshift[:],
                        scale=1.0,
                    )
                    # num extraction (diagonal j==k of the positive cols); the
                    # mul goes on the DVE queue before the exp-dependent
                    # tot-reduce so the PSUM release stays off the exp chain
                    scr = sp.tile([128, 2, PB], dt.float32, tag="ttr", name="ttr_o")
                    nc.vector.tensor_mul(
                        scr[:, 0:nb, :], P[:, 0:nb, 0:PB], maskI_t[:, 0:nb, :]
                    )
                    nc.vector.tensor_reduce(
                        num_all[:, 2 * m : 2 * m + nb],
                        scr[:, 0:nb, :],
                        axis=mybir.AxisListType.X,
                        op=mybir.AluOpType.add,
                    )
                    with nc.allow_low_precision(reason="bf16 tot validated <1e-5"):
                        nc.vector.tensor_reduce(
                            tot_all[:, 2 * m : 2 * m + nb],
                            E[:, 0:nb, :],
                            axis=mybir.AxisListType.X,
                            op=mybir.AluOpType.add,
                        )

                # --- final reduction ------------------------------------------
                # row-sum of num (DVE) runs concurrently with exp(num-50) (ACT)
                numsum = ap_.tile([128, 1], dt.float32)
                nc.vector.tensor_reduce(
                    numsum[:],
                    num_all[:, :NV],
                    axis=mybir.AxisListType.X,
                    op=mybir.AluOpType.add,
                )
                # fold in the positive term for all valid tiles: tot += exp(num-50)
                en_t = ap_.tile([128, NV], dt.bfloat16)
                nc.scalar.activation(
                    out=en_t[:],
                    in_=num_all[:, :NV],
                    func=mybir.ActivationFunctionType.Exp,
                    bias=negshift[:],
                    scale=1.0,
                )
                nc.vector.tensor_add(tot_all[:, :NV], tot_all[:, :NV], en_t[:])
                # ln(tot * 2^-32) keeps the ACT-ln input within its valid range;
                # +32*ln2 is restored on the host.  accum_out row-sums the ln.
                Lt = ap_.tile([128, NV], dt.float32)
                lnsum = ap_.tile([128, 1], dt.float32)
                nc.scalar.activation(
                    out=Lt[:],
                    in_=tot_all[:, :NV],
                    func=mybir.ActivationFunctionType.Ln,
                    scale=float(2.0**-32),
                    accum_out=lnsum[:],
                )
                rs = ap_.tile([128, 1], dt.float32)
                nc.vector.tensor_sub(rs[:], lnsum[:], numsum[:])
                psf = scps.tile([1, 1], dt.float32, name="P")
                nc.tensor.matmul(psf[:], lhsT=rs[:], rhs=pmask_t[:])
                outsb = ap_.tile([1, 1], dt.float32)
                nc.scalar.copy(out=outsb[:], in_=psf[:])
                nc.sync.dma_start(out[:], outsb[:])

    nc.compile()
    return nc


def prep_inputs(latent, W, samps):
    """Host-side sharding + layout marshalling. Returns per-core input maps."""
    latent = np.asarray(latent, dtype=np.float32)
    W = np.asarray(W, dtype=np.float32)
    samps = np.asarray(samps).astype(np.int64).reshape(N, Tp, M)

    lat8_all = latent.reshape(N * T, C).astype(FP8)
    # wT8[p, b, h, j] = W[b*128 + j, h*128 + p]
    wT8 = np.ascontiguousarray(
        W.astype(FP8).reshape(2 * K, 128, 2, 128).transpose(3, 0, 2, 1)
    )
    pmask = ((np.arange(128) % 32) < K).astype(np.float32).reshape(128, 1)
    k_arr = np.arange(128) % 32
    maskD = (
        (np.arange(PB)[None, :] == k_arr[:, None]) & (k_arr < K)[:, None]
    ).astype(np.float32)
    maskI4 = np.ascontiguousarray(np.tile(maskD, (1, 4)))

    win_idx = 1 + np.arange(Tp)[:, None] + np.arange(PB)[None, :]  # (Tp, PB)
    in_maps = []
    for n in range(N):
        lat8_n = lat8_all[n * T : (n + 1) * T]  # (T, C) fp8
        latT8 = np.ascontiguousarray(lat8_n.reshape(T, 2, 128).transpose(2, 1, 0))
        # stream block per (h, t): 12 positive cols ++ 128 gathered negatives
        blk = np.empty((Tp, FB, C), dtype=FP8)
        blk[:, :PB] = lat8_n[win_idx]  # (Tp, PB, C)
        blk[:, PB:] = lat8_all[samps[n]]  # (Tp, M, C)
        # phi8[p, t, h, j] = blk[t, j, h*128+p]
        phi8 = blk.reshape(Tp, FB, 2, 128).transpose(3, 0, 2, 1)
        in_maps.append(
            {
                "phi8": np.ascontiguousarray(phi8.reshape(128, 2 * Tp * FB)),
                "latT8": latT8,
                "wT8": wT8,
                "pmask": pmask,
                "maskI": maskI4,
            }
        )
    return in_maps


_NC_CACHE = None


def kernel(latent, W, samps):
    global _NC_CACHE
    from concourse import bass_utils

    if _NC_CACHE is None:
        _NC_CACHE = build_bass()
    nc = _NC_CACHE
    in_maps = prep_inputs(latent, W, samps)
    res = bass_utils.run_bass_kernel_spmd(nc, in_maps, core_ids=list(range(N)))
    partial = sum(float(r["out"][0, 0]) for r in res.results)
    import math

    return np.float32(partial / DENOM + SHIFT + 32.0 * math.log(2.0))


# revision 17
# speedup vs baseline: 1.5899x; 1.0315x over previous
"""CPC loss kernel for Trainium2 (8 NeuronCores, SPMD data-parallel over batch N).

Math (per batch element n, handled by core n):
  Az[t]   = W @ latent[n, t]            (K*C = 3072 outputs per position)
  scores[t, k, m] = phi[s_{t,m}] . Az[t, k]   (M=128 negatives per position)
  num[t, k]       = latent[n, 1+t+k] . Az[t, k]
  loss = mean over (n, t<500, k) of log(sum_m exp(scores) + exp(num)) - num

Device strategy per core (v2: DoubleRow score matmuls, 4-bank PSUM megatiles):
  - Host materializes the per-position rhs stream in fp8-e4m3 in the
    [c-half-major] layout the PE needs: phi[p, h, t, 0:12] are the positive
    latent cols (diagonal j==k extracted later), phi[p, h, t, 12:140] the
    gathered negatives.  Only the 500 real positions are streamed (~17.5 MB),
    in 17 chunks with a tiny final chunk so the drain tail is short.
  - AzT is computed via fp8 DoubleRow matmuls in two t-halves (so the second
    half's work overlaps the first score tiles) and stored fp8 in SBUF as
    azk[hh, k, t]; the score lhsT [128, 2, 32] is a strided view of it.
  - Scores: per position ONE DoubleRow matmul (contracts both c-halves) of
    the [128, 2, 140] stream block into a 4-bank PSUM tile (16 positions per
    tile, packed 4-per-bank via tile_position).  Per tile: one ACT
    exp(x-50) over [128, 4, 128] (bf16 out), one DVE masked mul extracting
    num from the 12 positive cols, and two DVE reduces.
  - Final: ln(tot*2^-32) with accum_out row-sum, minus the num row-sum,
    masked partition-sum via 1-col matmul.
Host: loss = sum(partials)/48000 + 50 + 32*ln(2).
"""

import sys, os

for _p in ("/opt/trn_rl_repo", "/root/.axon_site/_ro/trn_rl_repo"):
    if _p not in sys.path:
        sys.path.append(_p)

import numpy as np
import ml_dtypes

import concourse.bass as bass
import concourse.bacc as bacc
import concourse.mybir as mybir
from concourse.tile import TileContext, add_dep_helper

BF16 = ml_dtypes.bfloat16
FP8 = ml_dtypes.float8_e4m3

N, T, C, K, M = 8, 512, 256, 12, 128
Tp = T - K  # 500 real positions
PB = 12  # positive cols per position (k = 0..11)
FB = PB + M  # 140 stream cols per (position, c-half)
SHIFT = 50.0  # fixed logsumexp shift; |scores| << SHIFT + 88 so exp never overflows
DENOM = N * Tp * K  # 48000
NTILE = 32  # score tiles: 31 full (16 positions) + 1 of 4 positions
NV = Tp // 4  # 125 valid tot columns (4 positions each)

# phi chunk t-ranges: 8 + 24 (early first megatile) + 14x32 + 16 + 4 (tiny
# last chunk -> short drain tail)
CHUNKS = (
    [(0, 8), (8, 32)]
    + [(32 * i, 32 * (i + 1)) for i in range(1, 15)]
    + [(480, 496), (496, 500)]
)


def build_bass():
    nc = bacc.Bacc(
        "TRN2",
        target_bir_lowering=False,
        debug=False,
        enable_asserts=False,
    )
    dt = mybir.dt
    DR = mybir.MatmulPerfMode.DoubleRow

    # phi8[p, t, h, 0:12]   = fp8(latent[n, 1+t+j, h*128 + p]) (positives)
    # phi8[p, t, h, 12:140] = fp8(latent[samps[n,t,m] // T, _ % T, h*128 + p])
    phi8 = nc.dram_tensor("phi8", [128, 2 * Tp * FB], dt.float8e4, kind="ExternalInput").ap()
    latT8 = nc.dram_tensor("latT8", [128, 2, T], dt.float8e4, kind="ExternalInput").ap()
    wT8 = nc.dram_tensor("wT8", [128, 2 * K, 2, 128], dt.float8e4, kind="ExternalInput").ap()
    pmask = nc.dram_tensor("pmask", [128, 1], dt.float32, kind="ExternalInput").ap()
    maskI = nc.dram_tensor("maskI", [128, 4 * PB], dt.float32, kind="ExternalInput").ap()
    out = nc.dram_tensor("out", [1, 1], dt.float32, kind="ExternalOutput").ap()
    phi8v = phi8.rearrange("p (t hh j) -> p t hh j", hh=2, j=FB)

    with TileContext(nc) as tc:
        with (
            tc.tile_pool(name="const", bufs=1) as cp,
            tc.tile_pool(name="phi", bufs=1) as pp,
            tc.tile_pool(name="scr", bufs=6) as sp,
            tc.tile_pool(name="acc", bufs=1) as ap_,
        ):
            # --- constants + small DMAs on the (otherwise idle) SWDGE ring ---
            negshift = cp.tile([128, 1], dt.float32)
            nc.vector.memset(negshift[:], -SHIFT)
            wtile = cp.tile([128, 256], dt.bfloat16)
            nc.vector.memset(wtile[:], 0.5)
            # preload both ACT table sets (exp, ln) so the first real exp /
            # final ln don't stall on a ~1.3us table load
            tldca = cp.tile([128, 1], dt.float32)
            tldcb = cp.tile([128, 1], dt.float32)
            nc.vector.memset(tldca[:], 1.0)
            nc.scalar.activation(out=tldcb[:], in_=tldca[:], func=mybir.ActivationFunctionType.Exp)
            nc.scalar.activation(out=tldcb[:], in_=tldca[:], func=mybir.ActivationFunctionType.Ln)

            pmask_t = cp.tile([128, 1], dt.float32)
            maskI_t = cp.tile([128, 4, PB], dt.float32)
            nc.gpsimd.dma_start(pmask_t[:], pmask[:])
            nc.gpsimd.dma_start(maskI_t[:], maskI[:].rearrange("p (s j) -> p s j", j=PB))

            # --- weight + latent loads first on the sync ring, then the phi
            # stream; everything is consumed in roughly this order -----------
            latT8_t = cp.tile([128, 2, T], dt.float8e4)
            nc.sync.dma_start(latT8_t[:], latT8[:])
            # wT8 in 6 chunks so the first Az matmuls start as soon as the
            # first k-pairs land
            wT8_t = cp.tile([128, 2 * K, 2, 128], dt.float8e4)
            for c in range(6):
                nc.sync.dma_start(wT8_t[:, 4 * c : 4 * c + 4], wT8[:, 4 * c : 4 * c + 4])

            # AzT store: azk[p, hh, k, t] = Az[k, hh*128+p, t] (fp8), k padded
            # to 32 with zeros so the pad output partitions produce zero scores
            azsb = ap_.tile([128, 2 * 32 * T], dt.float8e4)
            azk = azsb.rearrange("p (hh k t) -> p hh k t", hh=2, k=32)
            # pad-k zeroing on the idle GPSIMD engine keeps the early DVE
            # queue free for the Az PSUM->SBUF copies
            nc.gpsimd.memzero(azk[:, 0, K:32, :])
            nc.gpsimd.memzero(azk[:, 1, K:32, :])

            # --- phi stream ---------------------------------------------------
            phi_t = pp.tile([128, 2 * Tp * FB], dt.float8e4)
            phi4 = phi_t.rearrange("p (t hh j) -> p t hh j", hh=2, j=FB)
            for (t0, t1) in CHUNKS:
                nc.sync.dma_start(phi4[:, t0:t1], phi8v[:, t0:t1])

            tot_all = ap_.tile([128, 128], dt.bfloat16)
            num_all = ap_.tile([128, 128], dt.float32)

            with tc.tile_pool(name="ps", bufs=4, space="PSUM") as scps:
                # dummy matmuls ramp the PE p-state out of its cold clock
                # while the weight DMAs are still in flight
                W0 = scps.tile([128, 2, 512], dt.float32, name="P")
                for _ in range(26):
                    nc.tensor.matmul(
                        W0[:, 0, 0:128],
                        lhsT=wtile[:, 0:128],
                        rhs=wtile[:, 128:256],
                        start=True,
                        stop=True,
                    )

                # --- Az phase ------------------------------------------------
                # AzT[kc, t] = sum_c' W[kc, c'] latent[n, t, c']; one DoubleRow
                # matmul per (k, h) contracts both c'-halves; both h-halves of
                # a k land in one 2-bank PSUM tile, copied out fp8 in one op.
                for k_ in range(K):
                    pa = scps.tile([128, 2, T], dt.float32, name="P")
                    for h_ in range(2):
                        nc.tensor.matmul(
                            pa[:, h_, :],
                            lhsT=wT8_t[:, 2 * k_ + h_, :, :],
                            rhs=latT8_t[:, :, :],
                            start=True,
                            stop=True,
                            perf_mode=DR,
                        )
                    if k_ % 2 == 0:
                        nc.scalar.copy(out=azk[:, :, k_, :], in_=pa[:, :, :])
                    else:
                        nc.vector.tensor_copy(out=azk[:, :, k_, :], in_=pa[:, :, :])

                # --- score megatiles: 8 positions, 2 PSUM banks ---------------
                NMT = Tp // 8 + 1  # 62 full + 1 single-bank (positions 496-499)

                def score_mt(m):
                    nb = 2 if m < NMT - 1 else 1
                    P = scps.tile([128, 2, 512], dt.float32, name="P")
                    for s in range(nb):
                        for q in range(4):
                            t = 8 * m + 4 * s + q
                            for h in range(2):
                                nc.tensor.matmul(
                                    P[32 * q : 32 * q + 32, s, 0:FB],
                                    lhsT=azk[:, h, :, t],
                                    rhs=phi4[:, t, h, :],
                                    start=(h == 0),
                                    stop=(h == 1),
                                    tile_position=(0, 32 * q),
                                )
                    # exp(scores - 50), bf16 out
                    E = sp.tile([128, 2, M], dt.bfloat16, tag="exp", name="exp_o")
                    nc.scalar.activation(
                        out=E[:, 0:nb, :],
                        in_=P[:, 0:nb, PB:FB],
                        func=mybir.ActivationFunctionType.Exp,
                        bias=negshift[:],
                        scale=1.0,
                    )
                    # num extraction (diagonal j==k of the positive cols); the
                    # mul goes on the DVE queue before the exp-dependent
                    # tot-reduce so the PSUM release stays off the exp chain
                    scr = sp.tile([128, 2, PB], dt.float32, tag="ttr", name="ttr_o")
                    nc.vector.tensor_mul(
                        scr[:, 0:nb, :], P[:, 0:nb, 0:PB], maskI_t[:, 0:nb, :]
                    )
                    nc.vector.tensor_reduce(
                        num_all[:, 2 * m : 2 * m + nb],
                        scr[:, 0:nb, :],
                        axis=mybir.AxisListType.X,
                        op=mybir.AluOpType.add,
                    )
                    with nc.allow_low_precision(reason="bf16 tot validated <1e-5"):
                        nc.vector.tensor_reduce(
                            tot_all[:, 2 * m : 2 * m + nb],
                            E[:, 0:nb, :],
                            axis=mybir.AxisListType.X,
                            op=mybir.AluOpType.add,
                        )

                for m in range(NMT - 1):
                    score_mt(m)

                # --- final reduction, mostly overlapped with the last megatile
                # (cols :124 are done while megatile 62's chunk streams in)
                numsumA = ap_.tile([128, 1], dt.float32)
                nc.vector.tensor_reduce(
                    numsumA[:],
                    num_all[:, : NV - 1],
                    axis=mybir.AxisListType.X,
                    op=mybir.AluOpType.add,
                )
                en_t = ap_.tile([128, NV], dt.bfloat16)
                nc.scalar.activation(
                    out=en_t[:, : NV - 1],
                    in_=num_all[:, : NV - 1],
                    func=mybir.ActivationFunctionType.Exp,
                    bias=negshift[:],
                    scale=1.0,
                )
                nc.vector.tensor_add(
                    tot_all[:, : NV - 1], tot_all[:, : NV - 1], en_t[:, : NV - 1]
                )

                score_mt(NMT - 1)

                nc.scalar.activation(
                    out=en_t[:, NV - 1 : NV],
                    in_=num_all[:, NV - 1 : NV],
                    func=mybir.ActivationFunctionType.Exp,
                    bias=negshift[:],
                    scale=1.0,
                )
                nc.vector.tensor_add(
                    tot_all[:, NV - 1 : NV], tot_all[:, NV - 1 : NV], en_t[:, NV - 1 : NV]
                )
                numsum = ap_.tile([128, 1], dt.float32)
                nc.vector.tensor_add(numsum[:], numsumA[:], num_all[:, NV - 1 : NV])
                # ln(tot * 2^-32) keeps the ACT-ln input within its valid range;
                # +32*ln2 is restored on the host.  accum_out row-sums the ln.
                Lt = ap_.tile([128, NV], dt.float32)
                lnsum = ap_.tile([128, 1], dt.float32)
                nc.scalar.activation(
                    out=Lt[:],
                    in_=tot_all[:, :NV],
                    func=mybir.ActivationFunctionType.Ln,
                    scale=float(2.0**-32),
                    accum_out=lnsum[:],
                )
                rs = ap_.tile([128, 1], dt.float32)
                nc.vector.tensor_sub(rs[:], lnsum[:], numsum[:])
                psf = scps.tile([1, 1], dt.float32, name="P")
                nc.tensor.matmul(psf[:], lhsT=rs[:], rhs=pmask_t[:])
                outsb = ap_.tile([1, 1], dt.float32)
                nc.scalar.copy(out=outsb[:], in_=psf[:])
                nc.sync.dma_start(out[:], outsb[:])

    nc.compile()
    return nc


def prep_inputs(latent, W, samps):
    """Host-side sharding + layout marshalling. Returns per-core input maps."""
    latent = np.asarray(latent, dtype=np.float32)
    W = np.asarray(W, dtype=np.float32)
    samps = np.asarray(samps).astype(np.int64).reshape(N, Tp, M)

    lat8_all = latent.reshape(N * T, C).astype(FP8)
    # wT8[p, b, h, j] = W[b*128 + j, h*128 + p]
    wT8 = np.ascontiguousarray(
        W.astype(FP8).reshape(2 * K, 128, 2, 128).transpose(3, 0, 2, 1)
    )
    pmask = ((np.arange(128) % 32) < K).astype(np.float32).reshape(128, 1)
    k_arr = np.arange(128) % 32
    maskD = (
        (np.arange(PB)[None, :] == k_arr[:, None]) & (k_arr < K)[:, None]
    ).astype(np.float32)
    maskI4 = np.ascontiguousarray(np.tile(maskD, (1, 4)))

    win_idx = 1 + np.arange(Tp)[:, None] + np.arange(PB)[None, :]  # (Tp, PB)
    in_maps = []
    for n in range(N):
        lat8_n = lat8_all[n * T : (n + 1) * T]  # (T, C) fp8
        latT8 = np.ascontiguousarray(lat8_n.reshape(T, 2, 128).transpose(2, 1, 0))
        # stream block per (h, t): 12 positive cols ++ 128 gathered negatives
        blk = np.empty((Tp, FB, C), dtype=FP8)
        blk[:, :PB] = lat8_n[win_idx]  # (Tp, PB, C)
        blk[:, PB:] = lat8_all[samps[n]]  # (Tp, M, C)
        # phi8[p, t, h, j] = blk[t, j, h*128+p]
        phi8 = blk.reshape(Tp, FB, 2, 128).transpose(3, 0, 2, 1)
        in_maps.append(
            {
                "phi8": np.ascontiguousarray(phi8.reshape(128, 2 * Tp * FB)),
                "latT8": latT8,
                "wT8": wT8,
                "pmask": pmask,
                "maskI": maskI4,
            }
        )
    return in_maps


_NC_CACHE = None


def kernel(latent, W, samps):
    global _NC_CACHE
    from concourse import bass_utils

    if _NC_CACHE is None:
        _NC_CACHE = build_bass()
    nc = _NC_CACHE
    in_maps = prep_inputs(latent, W, samps)
    res = bass_utils.run_bass_kernel_spmd(nc, in_maps, core_ids=list(range(N)))
    partial = sum(float(r["out"][0, 0]) for r in res.results)
    import math

    return np.float32(partial / DENOM + SHIFT + 32.0 * math.log(2.0))


# revision 19
# speedup vs baseline: 1.6036x; 1.0086x over previous
"""CPC loss kernel for Trainium2 (8 NeuronCores, SPMD data-parallel over batch N).

Math (per batch element n, handled by core n):
  Az[t]   = W @ latent[n, t]            (K*C = 3072 outputs per position)
  scores[t, k, m] = phi[s_{t,m}] . Az[t, k]   (M=128 negatives per position)
  num[t, k]       = latent[n, 1+t+k] . Az[t, k]
  loss = mean over (n, t<500, k) of log(sum_m exp(scores) + exp(num)) - num

Device strategy per core (v2: DoubleRow score matmuls, 4-bank PSUM megatiles):
  - Host materializes the per-position rhs stream in fp8-e4m3 in the
    [c-half-major] layout the PE needs: phi[p, h, t, 0:12] are the positive
    latent cols (diagonal j==k extracted later), phi[p, h, t, 12:140] the
    gathered negatives.  Only the 500 real positions are streamed (~17.5 MB),
    in 17 chunks with a tiny final chunk so the drain tail is short.
  - AzT is computed via fp8 DoubleRow matmuls in two t-halves (so the second
    half's work overlaps the first score tiles) and stored fp8 in SBUF as
    azk[hh, k, t]; the score lhsT [128, 2, 32] is a strided view of it.
  - Scores: per position ONE DoubleRow matmul (contracts both c-halves) of
    the [128, 2, 140] stream block into a 4-bank PSUM tile (16 positions per
    tile, packed 4-per-bank via tile_position).  Per tile: one ACT
    exp(x-50) over [128, 4, 128] (bf16 out), one DVE masked mul extracting
    num from the 12 positive cols, and two DVE reduces.
  - Final: ln(tot*2^-32) with accum_out row-sum, minus the num row-sum,
    masked partition-sum via 1-col matmul.
Host: loss = sum(partials)/48000 + 50 + 32*ln(2).
"""

import sys, os

for _p in ("/opt/trn_rl_repo", "/root/.axon_site/_ro/trn_rl_repo"):
    if _p not in sys.path:
        sys.path.append(_p)

import numpy as np
import ml_dtypes

import concourse.bass as bass
import concourse.bacc as bacc
import concourse.mybir as mybir
from concourse.tile import TileContext, add_dep_helper

BF16 = ml_dtypes.bfloat16
FP8 = ml_dtypes.float8_e4m3

N, T, C, K, M = 8, 512, 256, 12, 128
Tp = T - K  # 500 real positions
PB = 12  # positive cols per position (k = 0..11)
FB = PB + M  # 140 stream cols per (position, c-half)
SHIFT = 50.0  # fixed logsumexp shift; |scores| << SHIFT + 88 so exp never overflows
DENOM = N * Tp * K  # 48000
NTILE = 32  # score tiles: 31 full (16 positions) + 1 of 4 positions
NV = Tp // 4  # 125 valid tot columns (4 positions each)

# phi chunk t-ranges: 8 + 24 (early first megatile) + 14x32 + 16 + 4 (tiny
# last chunk -> short drain tail)
CHUNKS = (
    [(0, 8), (8, 32)]
    + [(32 * i, 32 * (i + 1)) for i in range(1, 15)]
    + [(480, 496), (496, 500)]
)


def build_bass():
    nc = bacc.Bacc(
        "TRN2",
        target_bir_lowering=False,
        debug=False,
        enable_asserts=False,
    )
    dt = mybir.dt
    DR = mybir.MatmulPerfMode.DoubleRow

    # phi8[p, t, h, 0:12]   = fp8(latent[n, 1+t+j, h*128 + p]) (positives)
    # phi8[p, t, h, 12:140] = fp8(latent[samps[n,t,m] // T, _ % T, h*128 + p])
    phi8 = nc.dram_tensor("phi8", [128, 2 * Tp * FB], dt.float8e4, kind="ExternalInput").ap()
    latT8 = nc.dram_tensor("latT8", [128, 2, T], dt.float8e4, kind="ExternalInput").ap()
    wT8 = nc.dram_tensor("wT8", [128, 2 * K, 2, 128], dt.float8e4, kind="ExternalInput").ap()
    pmask = nc.dram_tensor("pmask", [128, 1], dt.float32, kind="ExternalInput").ap()
    maskI = nc.dram_tensor("maskI", [128, 4 * PB], dt.float32, kind="ExternalInput").ap()
    out = nc.dram_tensor("out", [1, 1], dt.float32, kind="ExternalOutput").ap()
    phi8v = phi8.rearrange("p (t hh j) -> p t hh j", hh=2, j=FB)

    with TileContext(nc) as tc:
        with (
            tc.tile_pool(name="const", bufs=1) as cp,
            tc.tile_pool(name="phi", bufs=1) as pp,
            tc.tile_pool(name="scr", bufs=8) as sp,
            tc.tile_pool(name="acc", bufs=1) as ap_,
        ):
            # --- constants + small DMAs on the (otherwise idle) SWDGE ring ---
            negshift = cp.tile([128, 1], dt.float32)
            nc.vector.memset(negshift[:], -SHIFT)
            wtile = cp.tile([128, 256], dt.bfloat16)
            nc.vector.memset(wtile[:], 0.5)
            # preload both ACT table sets (exp, ln) so the first real exp /
            # final ln don't stall on a ~1.3us table load
            tldca = cp.tile([128, 1], dt.float32)
            tldcb = cp.tile([128, 1], dt.float32)
            nc.vector.memset(tldca[:], 1.0)
            nc.scalar.activation(out=tldcb[:], in_=tldca[:], func=mybir.ActivationFunctionType.Exp)
            nc.scalar.activation(out=tldcb[:], in_=tldca[:], func=mybir.ActivationFunctionType.Ln)

            pmask_t = cp.tile([128, 1], dt.float32)
            maskI_t = cp.tile([128, 4, PB], dt.float32)
            nc.gpsimd.dma_start(pmask_t[:], pmask[:])
            nc.gpsimd.dma_start(maskI_t[:], maskI[:].rearrange("p (s j) -> p s j", j=PB))

            # --- weight + latent loads first on the sync ring, then the phi
            # stream; everything is consumed in roughly this order -----------
            latT8_t = cp.tile([128, 2, T], dt.float8e4)
            nc.sync.dma_start(latT8_t[:], latT8[:])
            # wT8 in 6 chunks so the first Az matmuls start as soon as the
            # first k-pairs land
            wT8_t = cp.tile([128, 2 * K, 2, 128], dt.float8e4)
            for c in range(6):
                nc.sync.dma_start(wT8_t[:, 4 * c : 4 * c + 4], wT8[:, 4 * c : 4 * c + 4])

            # AzT store: azk[p, hh, k, t] = Az[k, hh*128+p, t] (fp8), k padded
            # to 32 with zeros so the pad output partitions produce zero scores
            azsb = ap_.tile([128, 2 * 32 * T], dt.float8e4)
            azk = azsb.rearrange("p (hh k t) -> p hh k t", hh=2, k=32)
            # pad-k zeroing on the idle GPSIMD engine keeps the early DVE
            # queue free for the Az PSUM->SBUF copies
            nc.gpsimd.memzero(azk[:, 0, K:32, :])
            nc.gpsimd.memzero(azk[:, 1, K:32, :])

            # --- phi stream ---------------------------------------------------
            phi_t = pp.tile([128, 2 * Tp * FB], dt.float8e4)
            phi4 = phi_t.rearrange("p (t hh j) -> p t hh j", hh=2, j=FB)
            for (t0, t1) in CHUNKS:
                nc.sync.dma_start(phi4[:, t0:t1], phi8v[:, t0:t1])

            tot_all = ap_.tile([128, 128], dt.bfloat16)
            num_all = ap_.tile([128, 128], dt.float32)

            with tc.tile_pool(name="ps", bufs=4, space="PSUM") as scps:
                # dummy matmuls ramp the PE p-state out of its cold clock
                # while the weight DMAs are still in flight
                W0 = scps.tile([128, 2, 512], dt.float32, name="P")
                for _ in range(26):
                    nc.tensor.matmul(
                        W0[:, 0, 0:128],
                        lhsT=wtile[:, 0:128],
                        rhs=wtile[:, 128:256],
                        start=True,
                        stop=True,
                    )

                # --- Az phase ------------------------------------------------
                # AzT[kc, t] = sum_c' W[kc, c'] latent[n, t, c']; one DoubleRow
                # matmul per (k, h) contracts both c'-halves; both h-halves of
                # a k land in one 2-bank PSUM tile, copied out fp8 in one op.
                for k_ in range(K):
                    pa = scps.tile([128, 2, T], dt.float32, name="P")
                    for h_ in range(2):
                        nc.tensor.matmul(
                            pa[:, h_, :],
                            lhsT=wT8_t[:, 2 * k_ + h_, :, :],
                            rhs=latT8_t[:, :, :],
                            start=True,
                            stop=True,
                            perf_mode=DR,
                        )
                    if k_ % 2 == 0:
                        nc.scalar.copy(out=azk[:, :, k_, :], in_=pa[:, :, :])
                    else:
                        nc.vector.tensor_copy(out=azk[:, :, k_, :], in_=pa[:, :, :])

                # keep the PE busy while the Az copies drain so the HAM clock
                # gate doesn't re-throttle before the first score matmuls
                W1 = scps.tile([128, 2, 512], dt.float32, name="P")
                for _ in range(30):
                    nc.tensor.matmul(
                        W1[:, 0, 0:128],
                        lhsT=wtile[:, 0:128],
                        rhs=wtile[:, 128:256],
                        start=True,
                        stop=True,
                    )

                # --- score megatiles: 8 positions, 2 PSUM banks ---------------
                NMT = Tp // 8 + 1  # 62 full + 1 single-bank (positions 496-499)

                def score_mt(m):
                    nb = 2 if m < NMT - 1 else 1
                    P = scps.tile([128, 2, 512], dt.float32, name="P")
                    for s in range(nb):
                        for q in range(4):
                            t = 8 * m + 4 * s + q
                            for h in range(2):
                                nc.tensor.matmul(
                                    P[32 * q : 32 * q + 32, s, 0:FB],
                                    lhsT=azk[:, h, :, t],
                                    rhs=phi4[:, t, h, :],
                                    start=(h == 0),
                                    stop=(h == 1),
                                    tile_position=(0, 32 * q),
                                )
                    # exp(scores - 50), bf16 out
                    E = sp.tile([128, 2, M], dt.bfloat16, tag="exp", name="exp_o")
                    nc.scalar.activation(
                        out=E[:, 0:nb, :],
                        in_=P[:, 0:nb, PB:FB],
                        func=mybir.ActivationFunctionType.Exp,
                        bias=negshift[:],
                        scale=1.0,
                    )
                    # num extraction (diagonal j==k of the positive cols); the
                    # mul goes on the DVE queue before the exp-dependent
                    # tot-reduce so the PSUM release stays off the exp chain
                    scr = sp.tile([128, 2, PB], dt.float32, tag="ttr", name="ttr_o")
                    nc.vector.tensor_mul(
                        scr[:, 0:nb, :], P[:, 0:nb, 0:PB], maskI_t[:, 0:nb, :]
                    )
                    nc.vector.tensor_reduce(
                        num_all[:, 2 * m : 2 * m + nb],
                        scr[:, 0:nb, :],
                        axis=mybir.AxisListType.X,
                        op=mybir.AluOpType.add,
                    )
                    with nc.allow_low_precision(reason="bf16 tot validated <1e-5"):
                        nc.vector.tensor_reduce(
                            tot_all[:, 2 * m : 2 * m + nb],
                            E[:, 0:nb, :],
                            axis=mybir.AxisListType.X,
                            op=mybir.AluOpType.add,
                        )

                for m in range(NMT - 1):
                    score_mt(m)

                # --- final reduction, mostly overlapped with the last megatile
                # (cols :124 are done while megatile 62's chunk streams in)
                numsumA = ap_.tile([128, 1], dt.float32)
                nc.vector.tensor_reduce(
                    numsumA[:],
                    num_all[:, : NV - 1],
                    axis=mybir.AxisListType.X,
                    op=mybir.AluOpType.add,
                )
                en_t = ap_.tile([128, NV], dt.bfloat16)
                nc.scalar.activation(
                    out=en_t[:, : NV - 1],
                    in_=num_all[:, : NV - 1],
                    func=mybir.ActivationFunctionType.Exp,
                    bias=negshift[:],
                    scale=1.0,
                )
                nc.vector.tensor_add(
                    tot_all[:, : NV - 1], tot_all[:, : NV - 1], en_t[:, : NV - 1]
                )

                score_mt(NMT - 1)

                nc.scalar.activation(
                    out=en_t[:, NV - 1 : NV],
                    in_=num_all[:, NV - 1 : NV],
                    func=mybir.ActivationFunctionType.Exp,
                    bias=negshift[:],
                    scale=1.0,
                )
                nc.vector.tensor_add(
                    tot_all[:, NV - 1 : NV], tot_all[:, NV - 1 : NV], en_t[:, NV - 1 : NV]
                )
                numsum = ap_.tile([128, 1], dt.float32)
                nc.vector.tensor_add(numsum[:], numsumA[:], num_all[:, NV - 1 : NV])
                # ln(tot * 2^-32) keeps the ACT-ln input within its valid range;
                # +32*ln2 is restored on the host.  accum_out row-sums the ln.
                Lt = ap_.tile([128, NV], dt.float32)
                lnsum = ap_.tile([128, 1], dt.float32)
                nc.scalar.activation(
                    out=Lt[:],
                    in_=tot_all[:, :NV],
                    func=mybir.ActivationFunctionType.Ln,
                    scale=float(2.0**-32),
                    accum_out=lnsum[:],
                )
                rs = ap_.tile([128, 1], dt.float32)
                nc.vector.tensor_sub(rs[:], lnsum[:], numsum[:])
                psf = scps.tile([1, 1], dt.float32, name="P")
                nc.tensor.matmul(psf[:], lhsT=rs[:], rhs=pmask_t[:])
                outsb = ap_.tile([1, 1], dt.float32)
                nc.scalar.copy(out=outsb[:], in_=psf[:])
                nc.sync.dma_start(out[:], outsb[:])

    nc.compile()
    return nc


def prep_inputs(latent, W, samps):
    """Host-side sharding + layout marshalling. Returns per-core input maps."""
    latent = np.asarray(latent, dtype=np.float32)
    W = np.asarray(W, dtype=np.float32)
    samps = np.asarray(samps).astype(np.int64).reshape(N, Tp, M)

    lat8_all = latent.reshape(N * T, C).astype(FP8)
    # wT8[p, b, h, j] = W[b*128 + j, h*128 + p]
    wT8 = np.ascontiguousarray(
        W.astype(FP8).reshape(2 * K, 128, 2, 128).transpose(3, 0, 2, 1)
    )
    pmask = ((np.arange(128) % 32) < K).astype(np.float32).reshape(128, 1)
    k_arr = np.arange(128) % 32
    maskD = (
        (np.arange(PB)[None, :] == k_arr[:, None]) & (k_arr < K)[:, None]
    ).astype(np.float32)
    maskI4 = np.ascontiguousarray(np.tile(maskD, (1, 4)))

    win_idx = 1 + np.arange(Tp)[:, None] + np.arange(PB)[None, :]  # (Tp, PB)
    in_maps = []
    for n in range(N):
        lat8_n = lat8_all[n * T : (n + 1) * T]  # (T, C) fp8
        latT8 = np.ascontiguousarray(lat8_n.reshape(T, 2, 128).transpose(2, 1, 0))
        # stream block per (h, t): 12 positive cols ++ 128 gathered negatives
        blk = np.empty((Tp, FB, C), dtype=FP8)
        blk[:, :PB] = lat8_n[win_idx]  # (Tp, PB, C)
        blk[:, PB:] = lat8_all[samps[n]]  # (Tp, M, C)
        # phi8[p, t, h, j] = blk[t, j, h*128+p]
        phi8 = blk.reshape(Tp, FB, 2, 128).transpose(3, 0, 2, 1)
        in_maps.append(
            {
                "phi8": np.ascontiguousarray(phi8.reshape(128, 2 * Tp * FB)),
                "latT8": latT8,
                "wT8": wT8,
                "pmask": pmask,
                "maskI": maskI4,
            }
        )
    return in_maps


_NC_CACHE = None


def kernel(latent, W, samps):
    global _NC_CACHE
    from concourse import bass_utils

    if _NC_CACHE is None:
        _NC_CACHE = build_bass()
    nc = _NC_CACHE
    in_maps = prep_inputs(latent, W, samps)
    res = bass_utils.run_bass_kernel_spmd(nc, in_maps, core_ids=list(range(N)))
    partial = sum(float(r["out"][0, 0]) for r in res.results)
    import math

    return np.float32(partial / DENOM + SHIFT + 32.0 * math.log(2.0))
